# revision 21
# baseline (speedup 1.0000x reference)
"""Block-circulant matvec (FFT linear layer) on 8 TRN2 cores — CRT + fp8 edition.

Math: the reference computes, per output block o,
    y[o, :] = sum_j IFFT(FFT(w[o,j]) * FFT(x[j])).real
which is a sum of length-128 circular convolutions:
    y[o, a] = sum_{j, b} w[o, j, b] * x[j, (a - b) mod 128]

CRT split (z^128-1 = (z^64-1)(z^64+1)): with
    w0 = w[..,:64] + w[..,64:]   (cyclic-64 part,   range [0,2), centered by 1)
    w1 = w[..,:64] - w[..,64:]   (negacyclic-64 part, already centered)
    x0 = (x[..,:64] + x[..,64:])/2,  x1 = (x[..,:64] - x[..,64:])/2
    y[a]    = P0[a] + P1[a] + S/2     (a < 64),   S = sum(x)
    y[a+64] = P0[a] - P1[a] + S/2
where P0 = (w0-1) (cyclic conv) x0 plus-const, P1 = w1 (negacyclic conv) x1.

Each half maps to matmuls like the baseline: per phase q and j-tile jt,
    PS[half][a, o] += XR[j', a]^T @ WT[j', o]
with XR an overlapping-window rotated-x tile (stationary, bf16, 64 cols) and
WT the weight slice (moving, float8e3 = e3m4, N=512). The two halves write
disjoint PSUM partition ranges (0-63 / 64-127), so the PE runs the pair as
concurrent col-group-tiled matmuls: W streams at 2x128 elem/cycle and the
weight bytes halve vs bf16 — both the PE time and the HBM time halve.

Centering w0 by 1 and quantizing the residual keeps the e3m4 step at 2^-6
over most of the mass; the removed constant contributes S/2, added on the
host. Measured rel err vs the fp32 reference: 3.7e-3 (gate 2e-2).

Timeline (measured): weight stream 8->20us at 390-460 GB/s (16-group
chunks = 8 KiB contiguous per-partition rows; bigger chunks make the
16-engine completion skew blow up the chunk semaphore lag, smaller rows
drop the per-packet HBM efficiency), warm col-tiled MM pairs chase each
chunk semaphore at 216ns/pair, tapered tail chunks (8,4,2,2) cut the
final data->semaphore receipt exposure, bf16 output store + parallel
ACT/DVE PSUM evacuation. ~24 N=256 warm-up matmuls bridge the HAM
clock-gate window so every real matmul issues at 2.4 GHz.

Sharding: 64 cyclic + 64 negacyclic phases split 8-per-core; the per-core
phase offset is folded into a host-side (anti)cyclic roll of the x window
buffers so the SPMD program is core-independent. Host sums the 8 partial
PSUM images and applies the butterfly + S/2.
"""

import numpy as np
import ml_dtypes

O_BLOCKS = 512
I_BLOCKS = 512
BLOCK = 128
HALF = 64
N_CORES = 8
Q_PER_CORE = HALF // N_CORES           # 8 phases per core per half
JT_TILES = I_BLOCKS // 128             # 4 contraction tiles
JH = JT_TILES * 2                      # jt*2 + half
N_GROUPS = Q_PER_CORE * JT_TILES * 2   # 64 matmul groups per core
CHUNK_GROUPS = (16, 12, 12, 12, 8, 2, 2)
assert sum(CHUNK_GROUPS) == N_GROUPS
N_WARMUP_MM = 12   # dummy matmuls (N=512) to lift the PE HAM clock-gate
WARM_N = 512

_BF16 = ml_dtypes.bfloat16
_F8E3 = ml_dtypes.float8_e3m4

_MODULE_CACHE = {}


def _build_module():
    import concourse.bacc as bacc
    import concourse.mybir as mybir
    from concourse import tile

    nc = bacc.Bacc(
        "TRN2",
        target_bir_lowering=False,
        debug=False,
        enable_asserts=False,
        enable_partition_id=False,
        num_devices=N_CORES,
    )

    xb2_d = nc.dram_tensor(
        "xb2", [128, JH, BLOCK], mybir.dt.bfloat16, kind="ExternalInput"
    )
    wt_d = nc.dram_tensor(
        "wt", [128, N_GROUPS, O_BLOCKS], mybir.dt.float8e3, kind="ExternalInput"
    )
    yt_d = nc.dram_tensor(
        "yt", [BLOCK, O_BLOCKS], mybir.dt.bfloat16, kind="ExternalOutput"
    )

    with tile.TileContext(nc) as tc:
        with (
            tc.tile_pool(name="xbp", bufs=1) as xbp,
            tc.tile_pool(name="wtp", bufs=len(CHUNK_GROUPS)) as wtp,
            tc.tile_pool(name="psp", bufs=2, space="PSUM") as psp,
            tc.tile_pool(name="outp", bufs=1) as outp,
            tc.tile_pool(name="scrp", bufs=1) as scrp,
        ):
            # PE warm-up: HAM clock gate holds the PE at 1.2 GHz until it has
            # been busy ~3.4us; dummy matmuls while the weight stream is in
            # flight let the real matmuls issue at 2.4 GHz.
            scr = scrp.tile([128, WARM_N], mybir.dt.bfloat16)
            nc.gpsimd.memset(scr[:], 0.0)
            ps_warm = psp.tile([BLOCK, O_BLOCKS], mybir.dt.float32)
            for _ in range(N_WARMUP_MM):
                nc.tensor.matmul(
                    ps_warm[:, :WARM_N], scr[:, :BLOCK], scr[:], start=True, stop=True
                )

            xb2_sb = xbp.tile([128, JH, BLOCK], mybir.dt.bfloat16)

            # The rotated-x stationary tiles are overlapping windows into
            # xb2: xr(q, jh)[p, a] = xb2[p, jh, q + a]. LDWEIGHTS reads the
            # window APs directly — no DVE staging copy needed.
            ps = psp.tile([BLOCK, O_BLOCKS], mybir.dt.float32)

            # Sync-ring FIFO order: chunk0, xb2, chunk1.. — the stream's END
            # (the critical path) moves earlier by one issue slot, and xb2
            # still lands long before its first consumer. Same-ring FIFO, so
            # the small transfer cannot be starved (unlike a second ring,
            # which round-robins at packet granularity - measured ~7us late).
            g0 = 0
            for ci, n_g in enumerate(CHUNK_GROUPS):
                wt_sb = wtp.tile([128, n_g, O_BLOCKS], mybir.dt.float8e3, tag="wchunk")
                nc.sync.dma_start(wt_sb[:], wt_d[:, g0 : g0 + n_g, :])
                if ci == 0:
                    nc.sync.dma_start(xb2_sb[:], xb2_d[:])
                for gi in range(n_g):
                    g = g0 + gi
                    half = g & 1
                    q, jh = divmod(g, JH)
                    ps_half = ps[HALF : 2 * HALF, :] if half else ps[:HALF, :]
                    nc.tensor.matmul(
                        ps_half,
                        xb2_sb[:, jh, q : q + HALF],
                        wt_sb[:, gi, :],
                        start=(g < 2),
                        stop=(g >= N_GROUPS - 2),
                    )
                g0 += n_g

            # evacuate PSUM in column halves on two engines concurrently
            # (ACT + DVE), then store on the two independent HWDGE rings so
            # issue and completion receipts also run in parallel
            out_sb = outp.tile([BLOCK, O_BLOCKS], mybir.dt.bfloat16)
            half_o = O_BLOCKS // 2
            nc.scalar.copy(out_sb[:, :half_o], ps[:, :half_o])
            nc.scalar.dma_start(yt_d[:, :half_o], out_sb[:, :half_o])
            nc.vector.tensor_copy(out_sb[:, half_o:], ps[:, half_o:])
            nc.sync.dma_start(yt_d[:, half_o:], out_sb[:, half_o:])

    nc.compile()
    return nc


def _get_module():
    if "nc" not in _MODULE_CACHE:
        _MODULE_CACHE["nc"] = _build_module()
    return _MODULE_CACHE["nc"]


def _prepare_inputs(x, cir_weights):
    xb = np.asarray(x, dtype=np.float32).reshape(I_BLOCKS, BLOCK)
    W = np.asarray(cir_weights, dtype=np.float32)

    # CRT halves
    x0 = (xb[:, :HALF] + xb[:, HALF:]) * 0.5          # [j, 64]
    x1 = (xb[:, :HALF] - xb[:, HALF:]) * 0.5
    w0c = (W[..., :HALF] + W[..., HALF:]) - 1.0       # centered cyclic part
    w1 = W[..., :HALF] - W[..., HALF:]
    d0q = w0c.astype(_F8E3)                           # [o, j, b]
    w1q = w1.astype(_F8E3)

    in_maps = []
    for c in range(N_CORES):
        # Group (q, jt, half) on core c handles phase b = 8c + 7 - q, so the
        # on-chip window walk (src offset q+a) sees ascending q. The window
        # buffers bake in the per-core shift t = m - (8c+7):
        #   cyclic:     C[j, m] = x0[j, t mod 64]
        #   negacyclic: N[j, m] = x1[j, t mod 64] * (-1)^floor(t/64)
        t = np.arange(BLOCK) - (N_CORES * c + Q_PER_CORE - 1)
        tm = t % HALF
        sgn = np.where((t // HALF) % 2 == 0, 1.0, -1.0).astype(np.float32)
        C = x0[:, tm]                                  # [j, 128]
        Nn = x1[:, tm] * sgn[None, :]
        C4 = C.reshape(JT_TILES, 128, BLOCK)           # [jt, j', m]
        N4 = Nn.reshape(JT_TILES, 128, BLOCK)
        xb2 = np.stack([C4, N4], axis=1)               # [jt, half, j', m]
        xb2 = np.ascontiguousarray(
            xb2.transpose(2, 0, 1, 3).reshape(128, JH, BLOCK)
        ).astype(_BF16)                                # [j', jh, m]

        qsl = slice(N_CORES * c, N_CORES * c + Q_PER_CORE)
        s0 = d0q[:, :, qsl][..., ::-1]                 # [o, j, q], q -> b=8c+7-q
        s1 = w1q[:, :, qsl][..., ::-1]
        s0r = s0.reshape(O_BLOCKS, JT_TILES, 128, Q_PER_CORE)   # [o, jt, j', q]
        s1r = s1.reshape(O_BLOCKS, JT_TILES, 128, Q_PER_CORE)
        st = np.stack([s0r, s1r], axis=0)              # [half, o, jt, j', q]
        wt = np.ascontiguousarray(
            st.transpose(3, 4, 2, 0, 1).reshape(128, N_GROUPS, O_BLOCKS)
        )                                              # [j', (q, jt, half), o]

        in_maps.append({"xb2": xb2, "wt": wt})
    return in_maps


def kernel(x, cir_weights):
    from concourse.bass_utils import run_bass_kernel_spmd

    nc = _get_module()
    in_maps = _prepare_inputs(x, cir_weights)
    res = run_bass_kernel_spmd(nc, in_maps, core_ids=list(range(N_CORES)))

    yt = np.zeros((BLOCK, O_BLOCKS), dtype=np.float64)
    for r in res.results:
        yt += np.asarray(r["yt"], dtype=np.float64)
    s_half = 0.5 * float(np.asarray(x, dtype=np.float64).sum())
    p0 = yt[:HALF]                                     # [64, 512] cyclic
    p1 = yt[HALF:]                                     # [64, 512] negacyclic
    y_low = p0 + p1 + s_half                           # a in [0, 64)
    y_high = p0 - p1 + s_half                          # a in [64, 128)
    ya = np.concatenate([y_low, y_high], axis=0)       # [a 128, o 512]
    return np.ascontiguousarray(ya.T).astype(np.float32).reshape(O_BLOCKS * BLOCK)



# revision 22
# speedup vs baseline: 1.0911x; 1.0911x over previous
"""Block-circulant matvec (FFT linear layer) on 8 TRN2 cores — CRT + fp8 edition.

Math: the reference computes, per output block o,
    y[o, :] = sum_j IFFT(FFT(w[o,j]) * FFT(x[j])).real
which is a sum of length-128 circular convolutions:
    y[o, a] = sum_{j, b} w[o, j, b] * x[j, (a - b) mod 128]

CRT split (z^128-1 = (z^64-1)(z^64+1)): with
    w0 = w[..,:64] + w[..,64:]   (cyclic-64 part,   range [0,2), centered by 1)
    w1 = w[..,:64] - w[..,64:]   (negacyclic-64 part, already centered)
    x0 = (x[..,:64] + x[..,64:])/2,  x1 = (x[..,:64] - x[..,64:])/2
    y[a]    = P0[a] + P1[a] + S/2     (a < 64),   S = sum(x)
    y[a+64] = P0[a] - P1[a] + S/2
where P0 = (w0-1) (cyclic conv) x0 plus-const, P1 = w1 (negacyclic conv) x1.

Each half maps to matmuls like the baseline: per phase q and j-tile jt,
    PS[half][a, o] += XR[j', a]^T @ WT[j', o]
with XR an overlapping-window rotated-x tile (stationary, bf16, 64 cols) and
WT the weight slice (moving, float8e3 = e3m4, N=512). The two halves write
disjoint PSUM partition ranges (0-63 / 64-127), so the PE runs the pair as
concurrent col-group-tiled matmuls: W streams at 2x128 elem/cycle and the
weight bytes halve vs bf16 — both the PE time and the HBM time halve.

Centering w0 by 1 and quantizing the residual keeps the e3m4 step at 2^-6
over most of the mass; the removed constant contributes S/2, added on the
host. Measured rel err vs the fp32 reference: 3.7e-3 (gate 2e-2).

Timeline (measured): weight stream 8->20us at 390-460 GB/s (16-group
chunks = 8 KiB contiguous per-partition rows; bigger chunks make the
16-engine completion skew blow up the chunk semaphore lag, smaller rows
drop the per-packet HBM efficiency), warm col-tiled MM pairs chase each
chunk semaphore at 216ns/pair, tapered tail chunks (8,4,2,2) cut the
final data->semaphore receipt exposure, bf16 output store + parallel
ACT/DVE PSUM evacuation. ~24 N=256 warm-up matmuls bridge the HAM
clock-gate window so every real matmul issues at 2.4 GHz.

Sharding: 64 cyclic + 64 negacyclic phases split 8-per-core; the per-core
phase offset is folded into a host-side (anti)cyclic roll of the x window
buffers so the SPMD program is core-independent. Host sums the 8 partial
PSUM images and applies the butterfly + S/2.
"""

import numpy as np
import ml_dtypes

O_BLOCKS = 512
I_BLOCKS = 512
BLOCK = 128
HALF = 64
N_CORES = 8
Q_PER_CORE = HALF // N_CORES           # 8 phases per core per half
JT_TILES = I_BLOCKS // 128             # 4 contraction tiles
JH = JT_TILES * 2                      # jt*2 + half
N_GROUPS = Q_PER_CORE * JT_TILES * 2   # 64 matmul groups per core
CHUNK_GROUPS = (16, 16, 16, 8, 4, 2, 2)
assert sum(CHUNK_GROUPS) == N_GROUPS
N_WARMUP_MM = 12   # dummy matmuls (N=512) to lift the PE HAM clock-gate
WARM_N = 512

_BF16 = ml_dtypes.bfloat16
_F8E3 = ml_dtypes.float8_e3m4

_MODULE_CACHE = {}


def _build_module():
    import concourse.bacc as bacc
    import concourse.mybir as mybir
    from concourse import tile

    nc = bacc.Bacc(
        "TRN2",
        target_bir_lowering=False,
        debug=False,
        enable_asserts=False,
        enable_partition_id=False,
        num_devices=N_CORES,
    )

    xb2_d = nc.dram_tensor(
        "xb2", [128, JH, BLOCK], mybir.dt.bfloat16, kind="ExternalInput"
    )
    wt_d = nc.dram_tensor(
        "wt", [128, N_GROUPS, O_BLOCKS], mybir.dt.float8e3, kind="ExternalInput"
    )
    yt_d = nc.dram_tensor(
        "yt", [BLOCK, O_BLOCKS], mybir.dt.bfloat16, kind="ExternalOutput"
    )

    with tile.TileContext(nc) as tc:
        with (
            tc.tile_pool(name="xbp", bufs=1) as xbp,
            tc.tile_pool(name="wtp", bufs=len(CHUNK_GROUPS)) as wtp,
            tc.tile_pool(name="psp", bufs=2, space="PSUM") as psp,
            tc.tile_pool(name="outp", bufs=1) as outp,
            tc.tile_pool(name="scrp", bufs=1) as scrp,
        ):
            # PE warm-up: HAM clock gate holds the PE at 1.2 GHz until it has
            # been busy ~3.4us; dummy matmuls while the weight stream is in
            # flight let the real matmuls issue at 2.4 GHz.
            scr = scrp.tile([128, WARM_N], mybir.dt.bfloat16)
            nc.gpsimd.memset(scr[:], 0.0)
            ps_warm = psp.tile([BLOCK, O_BLOCKS], mybir.dt.float32)
            for _ in range(N_WARMUP_MM):
                nc.tensor.matmul(
                    ps_warm[:, :WARM_N], scr[:, :BLOCK], scr[:], start=True, stop=True
                )

            xb2_sb = xbp.tile([128, JH, BLOCK], mybir.dt.bfloat16)

            # The rotated-x stationary tiles are overlapping windows into
            # xb2: xr(q, jh)[p, a] = xb2[p, jh, q + a]. LDWEIGHTS reads the
            # window APs directly — no DVE staging copy needed.
            ps = psp.tile([BLOCK, O_BLOCKS], mybir.dt.float32)

            # Sync-ring FIFO order: chunk0, xb2, chunk1.. — the stream's END
            # (the critical path) moves earlier by one issue slot, and xb2
            # still lands long before its first consumer. Same-ring FIFO, so
            # the small transfer cannot be starved (unlike a second ring,
            # which round-robins at packet granularity - measured ~7us late).
            g0 = 0
            for ci, n_g in enumerate(CHUNK_GROUPS):
                wt_sb = wtp.tile([128, n_g, O_BLOCKS], mybir.dt.float8e3, tag="wchunk")
                nc.sync.dma_start(wt_sb[:], wt_d[:, g0 : g0 + n_g, :])
                if ci == 0:
                    nc.sync.dma_start(xb2_sb[:], xb2_d[:])
                for gi in range(n_g):
                    g = g0 + gi
                    half = g & 1
                    q, jh = divmod(g, JH)
                    ps_half = ps[HALF : 2 * HALF, :] if half else ps[:HALF, :]
                    nc.tensor.matmul(
                        ps_half,
                        xb2_sb[:, jh, q : q + HALF],
                        wt_sb[:, gi, :],
                        start=(g < 2),
                        stop=(g >= N_GROUPS - 2),
                    )
                g0 += n_g

            # evacuate PSUM in column halves on two engines concurrently
            # (ACT + DVE), then store on the two independent HWDGE rings so
            # issue and completion receipts also run in parallel
            out_sb = outp.tile([BLOCK, O_BLOCKS], mybir.dt.bfloat16)
            half_o = O_BLOCKS // 2
            nc.scalar.copy(out_sb[:, :half_o], ps[:, :half_o])
            nc.scalar.dma_start(yt_d[:, :half_o], out_sb[:, :half_o])
            nc.vector.tensor_copy(out_sb[:, half_o:], ps[:, half_o:])
            nc.sync.dma_start(yt_d[:, half_o:], out_sb[:, half_o:])

    nc.compile()
    return nc


def _get_module():
    if "nc" not in _MODULE_CACHE:
        _MODULE_CACHE["nc"] = _build_module()
    return _MODULE_CACHE["nc"]


def _prepare_inputs(x, cir_weights):
    xb = np.asarray(x, dtype=np.float32).reshape(I_BLOCKS, BLOCK)
    W = np.asarray(cir_weights, dtype=np.float32)

    # CRT halves
    x0 = (xb[:, :HALF] + xb[:, HALF:]) * 0.5          # [j, 64]
    x1 = (xb[:, :HALF] - xb[:, HALF:]) * 0.5
    w0c = (W[..., :HALF] + W[..., HALF:]) - 1.0       # centered cyclic part
    w1 = W[..., :HALF] - W[..., HALF:]
    d0q = w0c.astype(_F8E3)                           # [o, j, b]
    w1q = w1.astype(_F8E3)

    in_maps = []
    for c in range(N_CORES):
        # Group (q, jt, half) on core c handles phase b = 8c + 7 - q, so the
        # on-chip window walk (src offset q+a) sees ascending q. The window
        # buffers bake in the per-core shift t = m - (8c+7):
        #   cyclic:     C[j, m] = x0[j, t mod 64]
        #   negacyclic: N[j, m] = x1[j, t mod 64] * (-1)^floor(t/64)
        t = np.arange(BLOCK) - (N_CORES * c + Q_PER_CORE - 1)
        tm = t % HALF
        sgn = np.where((t // HALF) % 2 == 0, 1.0, -1.0).astype(np.float32)
        C = x0[:, tm]                                  # [j, 128]
        Nn = x1[:, tm] * sgn[None, :]
        C4 = C.reshape(JT_TILES, 128, BLOCK)           # [jt, j', m]
        N4 = Nn.reshape(JT_TILES, 128, BLOCK)
        xb2 = np.stack([C4, N4], axis=1)               # [jt, half, j', m]
        xb2 = np.ascontiguousarray(
            xb2.transpose(2, 0, 1, 3).reshape(128, JH, BLOCK)
        ).astype(_BF16)                                # [j', jh, m]

        qsl = slice(N_CORES * c, N_CORES * c + Q_PER_CORE)
        s0 = d0q[:, :, qsl][..., ::-1]                 # [o, j, q], q -> b=8c+7-q
        s1 = w1q[:, :, qsl][..., ::-1]
        s0r = s0.reshape(O_BLOCKS, JT_TILES, 128, Q_PER_CORE)   # [o, jt, j', q]
        s1r = s1.reshape(O_BLOCKS, JT_TILES, 128, Q_PER_CORE)
        st = np.stack([s0r, s1r], axis=0)              # [half, o, jt, j', q]
        wt = np.ascontiguousarray(
            st.transpose(3, 4, 2, 0, 1).reshape(128, N_GROUPS, O_BLOCKS)
        )                                              # [j', (q, jt, half), o]

        in_maps.append({"xb2": xb2, "wt": wt})
    return in_maps


def kernel(x, cir_weights):
    from concourse.bass_utils import run_bass_kernel_spmd

    nc = _get_module()
    in_maps = _prepare_inputs(x, cir_weights)
    res = run_bass_kernel_spmd(nc, in_maps, core_ids=list(range(N_CORES)))

    yt = np.zeros((BLOCK, O_BLOCKS), dtype=np.float64)
    for r in res.results:
        yt += np.asarray(r["yt"], dtype=np.float64)
    s_half = 0.5 * float(np.asarray(x, dtype=np.float64).sum())
    p0 = yt[:HALF]                                     # [64, 512] cyclic
    p1 = yt[HALF:]                                     # [64, 512] negacyclic
    y_low = p0 + p1 + s_half                           # a in [0, 64)
    y_high = p0 - p1 + s_half                          # a in [64, 128)
    ya = np.concatenate([y_low, y_high], axis=0)       # [a 128, o 512]
    return np.ascontiguousarray(ya.T).astype(np.float32).reshape(O_BLOCKS * BLOCK)



# revision 24
# speedup vs baseline: 1.1011x; 1.0092x over previous
"""Block-circulant matvec (FFT linear layer) on 8 TRN2 cores — CRT + fp8 edition.

Math: the reference computes, per output block o,
    y[o, :] = sum_j IFFT(FFT(w[o,j]) * FFT(x[j])).real
which is a sum of length-128 circular convolutions:
    y[o, a] = sum_{j, b} w[o, j, b] * x[j, (a - b) mod 128]

CRT split (z^128-1 = (z^64-1)(z^64+1)): with
    w0 = w[..,:64] + w[..,64:]   (cyclic-64 part,   range [0,2), centered by 1)
    w1 = w[..,:64] - w[..,64:]   (negacyclic-64 part, already centered)
    x0 = (x[..,:64] + x[..,64:])/2,  x1 = (x[..,:64] - x[..,64:])/2
    y[a]    = P0[a] + P1[a] + S/2     (a < 64),   S = sum(x)
    y[a+64] = P0[a] - P1[a] + S/2
where P0 = (w0-1) (cyclic conv) x0 plus-const, P1 = w1 (negacyclic conv) x1.

Each half maps to matmuls like the baseline: per phase q and j-tile jt,
    PS[half][a, o] += XR[j', a]^T @ WT[j', o]
with XR an overlapping-window rotated-x tile (stationary, bf16, 64 cols) and
WT the weight slice (moving, float8e3 = e3m4, N=512). The two halves write
disjoint PSUM partition ranges (0-63 / 64-127), so the PE runs the pair as
concurrent col-group-tiled matmuls: W streams at 2x128 elem/cycle and the
weight bytes halve vs bf16 — both the PE time and the HBM time halve.

Centering w0 by 1 and quantizing the residual keeps the e3m4 step at 2^-6
over most of the mass; the removed constant contributes S/2, added on the
host. Measured rel err vs the fp32 reference: 3.7e-3 (gate 2e-2).

Timeline (measured): weight stream 8->20us at 390-460 GB/s (16-group
chunks = 8 KiB contiguous per-partition rows; bigger chunks make the
16-engine completion skew blow up the chunk semaphore lag, smaller rows
drop the per-packet HBM efficiency), warm col-tiled MM pairs chase each
chunk semaphore at 216ns/pair, tapered tail chunks (8,4,2,2) cut the
final data->semaphore receipt exposure, bf16 output store + parallel
ACT/DVE PSUM evacuation. 12 N=512 warm-up matmuls bridge the HAM
clock-gate window so every real matmul issues at 2.4 GHz.

Sharding: 64 cyclic + 64 negacyclic phases split 8-per-core; the per-core
phase offset is folded into a host-side (anti)cyclic roll of the x window
buffers so the SPMD program is core-independent. Host sums the 8 partial
PSUM images and applies the butterfly + S/2.
"""

import numpy as np
import ml_dtypes

O_BLOCKS = 512
I_BLOCKS = 512
BLOCK = 128
HALF = 64
N_CORES = 8
Q_PER_CORE = HALF // N_CORES           # 8 phases per core per half
JT_TILES = I_BLOCKS // 128             # 4 contraction tiles
JH = JT_TILES * 2                      # jt*2 + half
N_GROUPS = Q_PER_CORE * JT_TILES * 2   # 64 matmul groups per core
XBW = 72                               # x-window buffer cols (q+a <= 70 used)
CHUNK_GROUPS = (16, 16, 16, 8, 4, 2, 2)
assert sum(CHUNK_GROUPS) == N_GROUPS
N_WARMUP_MM = 12   # dummy matmuls (N=512) to lift the PE HAM clock-gate
WARM_N = 512

_BF16 = ml_dtypes.bfloat16
_F8E3 = ml_dtypes.float8_e3m4

_MODULE_CACHE = {}


def _build_module():
    import concourse.bacc as bacc
    import concourse.mybir as mybir
    from concourse import tile

    nc = bacc.Bacc(
        "TRN2",
        target_bir_lowering=False,
        debug=False,
        enable_asserts=False,
        enable_partition_id=False,
        num_devices=N_CORES,
    )

    xb2_d = nc.dram_tensor(
        "xb2", [128, JH, XBW], mybir.dt.bfloat16, kind="ExternalInput"
    )
    wt_d = nc.dram_tensor(
        "wt", [128, N_GROUPS, O_BLOCKS], mybir.dt.float8e3, kind="ExternalInput"
    )
    yt_d = nc.dram_tensor(
        "yt", [BLOCK, O_BLOCKS], mybir.dt.bfloat16, kind="ExternalOutput"
    )

    with tile.TileContext(nc) as tc:
        with (
            tc.tile_pool(name="xbp", bufs=1) as xbp,
            tc.tile_pool(name="wtp", bufs=len(CHUNK_GROUPS)) as wtp,
            tc.tile_pool(name="psp", bufs=2, space="PSUM") as psp,
            tc.tile_pool(name="outp", bufs=1) as outp,
            tc.tile_pool(name="scrp", bufs=1) as scrp,
        ):
            # PE warm-up: HAM clock gate holds the PE at 1.2 GHz until it has
            # been busy ~3.4us; dummy matmuls while the weight stream is in
            # flight let the real matmuls issue at 2.4 GHz.
            scr = scrp.tile([128, WARM_N], mybir.dt.bfloat16)
            nc.gpsimd.memset(scr[:], 0.0)
            ps_warm = psp.tile([BLOCK, O_BLOCKS], mybir.dt.float32)
            for _ in range(N_WARMUP_MM):
                nc.tensor.matmul(
                    ps_warm[:, :WARM_N], scr[:, :BLOCK], scr[:], start=True, stop=True
                )

            xb2_sb = xbp.tile([128, JH, XBW], mybir.dt.bfloat16)

            # The rotated-x stationary tiles are overlapping windows into
            # xb2: xr(q, jh)[p, a] = xb2[p, jh, q + a]. LDWEIGHTS reads the
            # window APs directly — no DVE staging copy needed.
            ps = psp.tile([BLOCK, O_BLOCKS], mybir.dt.float32)

            # Sync-ring FIFO order: chunk0, xb2, chunk1.. — the stream's END
            # (the critical path) moves earlier by one issue slot, and xb2
            # still lands long before its first consumer. Same-ring FIFO, so
            # the small transfer cannot be starved (unlike a second ring,
            # which round-robins at packet granularity - measured ~7us late).
            g0 = 0
            for ci, n_g in enumerate(CHUNK_GROUPS):
                wt_sb = wtp.tile([128, n_g, O_BLOCKS], mybir.dt.float8e3, tag="wchunk")
                nc.sync.dma_start(wt_sb[:], wt_d[:, g0 : g0 + n_g, :])
                if ci == 0:
                    nc.sync.dma_start(xb2_sb[:], xb2_d[:])
                for gi in range(n_g):
                    g = g0 + gi
                    half = g & 1
                    q, jh = divmod(g, JH)
                    ps_half = ps[HALF : 2 * HALF, :] if half else ps[:HALF, :]
                    nc.tensor.matmul(
                        ps_half,
                        xb2_sb[:, jh, q : q + HALF],
                        wt_sb[:, gi, :],
                        start=(g < 2),
                        stop=(g >= N_GROUPS - 2),
                    )
                g0 += n_g

            # evacuate PSUM in column halves on two engines concurrently
            # (ACT + DVE), then store on the two independent HWDGE rings so
            # issue and completion receipts also run in parallel
            out_sb = outp.tile([BLOCK, O_BLOCKS], mybir.dt.bfloat16)
            half_o = O_BLOCKS // 2
            nc.scalar.copy(out_sb[:, :half_o], ps[:, :half_o])
            nc.scalar.dma_start(yt_d[:, :half_o], out_sb[:, :half_o])
            nc.vector.tensor_copy(out_sb[:, half_o:], ps[:, half_o:])
            nc.sync.dma_start(yt_d[:, half_o:], out_sb[:, half_o:])

    nc.compile()
    return nc


def _get_module():
    if "nc" not in _MODULE_CACHE:
        _MODULE_CACHE["nc"] = _build_module()
    return _MODULE_CACHE["nc"]


def _prepare_inputs(x, cir_weights):
    xb = np.asarray(x, dtype=np.float32).reshape(I_BLOCKS, BLOCK)
    W = np.asarray(cir_weights, dtype=np.float32)

    # CRT halves
    x0 = (xb[:, :HALF] + xb[:, HALF:]) * 0.5          # [j, 64]
    x1 = (xb[:, :HALF] - xb[:, HALF:]) * 0.5
    w0c = (W[..., :HALF] + W[..., HALF:]) - 1.0       # centered cyclic part
    w1 = W[..., :HALF] - W[..., HALF:]
    d0q = w0c.astype(_F8E3)                           # [o, j, b]
    w1q = w1.astype(_F8E3)

    in_maps = []
    for c in range(N_CORES):
        # Group (q, jt, half) on core c handles phase b = 8c + 7 - q, so the
        # on-chip window walk (src offset q+a) sees ascending q. The window
        # buffers bake in the per-core shift t = m - (8c+7):
        #   cyclic:     C[j, m] = x0[j, t mod 64]
        #   negacyclic: N[j, m] = x1[j, t mod 64] * (-1)^floor(t/64)
        t = np.arange(XBW) - (N_CORES * c + Q_PER_CORE - 1)
        tm = t % HALF
        sgn = np.where((t // HALF) % 2 == 0, 1.0, -1.0).astype(np.float32)
        C = x0[:, tm]                                  # [j, 72]
        Nn = x1[:, tm] * sgn[None, :]
        C4 = C.reshape(JT_TILES, 128, XBW)             # [jt, j', m]
        N4 = Nn.reshape(JT_TILES, 128, XBW)
        xb2 = np.stack([C4, N4], axis=1)               # [jt, half, j', m]
        xb2 = np.ascontiguousarray(
            xb2.transpose(2, 0, 1, 3).reshape(128, JH, XBW)
        ).astype(_BF16)                                # [j', jh, m]

        qsl = slice(N_CORES * c, N_CORES * c + Q_PER_CORE)
        s0 = d0q[:, :, qsl][..., ::-1]                 # [o, j, q], q -> b=8c+7-q
        s1 = w1q[:, :, qsl][..., ::-1]
        s0r = s0.reshape(O_BLOCKS, JT_TILES, 128, Q_PER_CORE)   # [o, jt, j', q]
        s1r = s1.reshape(O_BLOCKS, JT_TILES, 128, Q_PER_CORE)
        st = np.stack([s0r, s1r], axis=0)              # [half, o, jt, j', q]
        wt = np.ascontiguousarray(
            st.transpose(3, 4, 2, 0, 1).reshape(128, N_GROUPS, O_BLOCKS)
        )                                              # [j', (q, jt, half), o]

        in_maps.append({"xb2": xb2, "wt": wt})
    return in_maps


def kernel(x, cir_weights):
    from concourse.bass_utils import run_bass_kernel_spmd

    nc = _get_module()
    in_maps = _prepare_inputs(x, cir_weights)
    res = run_bass_kernel_spmd(nc, in_maps, core_ids=list(range(N_CORES)))

    yt = np.zeros((BLOCK, O_BLOCKS), dtype=np.float64)
    for r in res.results:
        yt += np.asarray(r["yt"], dtype=np.float64)
    s_half = 0.5 * float(np.asarray(x, dtype=np.float64).sum())
    p0 = yt[:HALF]                                     # [64, 512] cyclic
    p1 = yt[HALF:]                                     # [64, 512] negacyclic
    y_low = p0 + p1 + s_half                           # a in [0, 64)
    y_high = p0 - p1 + s_half                          # a in [64, 128)
    ya = np.concatenate([y_low, y_high], axis=0)       # [a 128, o 512]
    return np.ascontiguousarray(ya.T).astype(np.float32).reshape(O_BLOCKS * BLOCK)

